# revision 2
# baseline (speedup 1.0000x reference)
"""Batched Viterbi decode (CRF inference) on 8 Trainium2 NeuronCores — v2.

Data-parallel over batch (64 seqs/core) with a PAIR-SPLIT layout: each
sequence occupies two adjacent SBUF partitions (2s owns next-tag pages
j=0..15, 2s+1 owns j=16..31), so every per-step O(L^2) DVE op runs on
[128, 512] instead of [64, 1024] — half the per-partition free size.
stream_shuffle (partition permute within 32-lane quadrants; pairs never
cross a quadrant) rebuilds the replicated 32-wide alpha each step and
replicates backpointers into a full-width ring on both partitions.

Forward step t (j-page-local, i = global prev-tag 0..31):
    sc[p,(jj,i)] = alpha_full[p,i] + transT_own[p,(jj,i)]   TT add (bcast AP)
    R = per-page running max of sc                          TTS scan (rstv reset)
    M_own[p,jj] = R[p,(jj,31)]; cand = M_own + e_own        TT add (strided)
    alpha_own = where(sm_t, cand, alpha_own)                copy_predicated
    alpha_full <- 2x stream_shuffle(alpha_own)
    ltt = [R < M_own bcast] (fp16); bp_own = sum_i ltt (u8) = first-argmax
    ring[t] <- 2x stream_shuffle(bp_own)                    (u8, both halves)

The per-step instruction stream is software-pipelined: the bp-side ops of
step t-1/t-2 are interleaved into step t's serial alpha chain so that every
same-engine RAW semaphore wait has >=195ns of independent work between
producer and consumer (sem update propagation window) and costs ~nothing.
lt and reduce are emitted as two halves each purely to create filler.

Backtrace: tag_t = ring[t+1][tag_{t+1}] via scalar_tensor_tensor gather
(iota==tag)*ring summed; ring read directly as u8 (mixed-dtype STT).

All f32 adds/compares run in the same order on the same values as the jax
reference -> bit-exact paths incl. first-argmax tie-breaking.
"""

import sys

for p in ("/opt/trn_rl_repo", "/opt/pypackages"):
    if p not in sys.path:
        sys.path.insert(0, p)

from contextlib import ExitStack

import numpy as np

import concourse.bass as bass
from concourse import mybir
from concourse.bass_utils import run_bass_kernel_spmd

A = mybir.AluOpType
DT = mybir.dt
AX = mybir.AxisListType

B, T, L = 512, 2048, 32
NCORES = 8
SEQ = B // NCORES          # 64 sequences per core
P = 2 * SEQ                # 128 partitions, 2 per sequence
H = L // 2                 # 16 own pages per partition
NEG = -1.0e30
CHUNK = 64                 # forward steps per emission DMA chunk
CE = CHUNK * H             # chunk elems per partition

EVEN = [q * 2 for q in range(16) for _ in (0, 1)]     # pair -> even member
ODD = [q * 2 + 1 for q in range(16) for _ in (0, 1)]  # pair -> odd member


def build_program(T_=T):
    assert T_ % CHUNK == 0
    nch = T_ // CHUNK

    nc = bass.Bass()
    nc.detect_race_conditions = False
    x = nc.declare_dram_parameter("x", [P, T_ * H], DT.float32, isOutput=False)
    trep = nc.declare_dram_parameter("trep", [P, H * L], DT.float32, isOutput=False)
    rstv = nc.declare_dram_parameter("rstv", [P, H * L], DT.float32, isOutput=False)
    iotf = nc.declare_dram_parameter("iotf", [P, L], DT.float32, isOutput=False)
    iotu = nc.declare_dram_parameter("iotu", [P, L], DT.uint8, isOutput=False)
    smt = nc.declare_dram_parameter("smt", [P, T_], DT.uint8, isOutput=False)
    nsmt = nc.declare_dram_parameter("nsmt", [P, T_], DT.uint8, isOutput=False)
    mt = nc.declare_dram_parameter("mt", [P, T_], DT.uint8, isOutput=False)
    pout = nc.declare_dram_parameter("paths", [P, T_], DT.int32, isOutput=True)

    with ExitStack() as ctx:
        e = ctx.enter_context
        trep_sb = e(nc.sbuf_tensor([P, H * L], DT.float32))
        rstv_sb = e(nc.sbuf_tensor([P, H * L], DT.float32))
        iotf_sb = e(nc.sbuf_tensor([P, L], DT.float32))
        iotu_sb = e(nc.sbuf_tensor([P, L], DT.uint8))
        sm_sb = e(nc.sbuf_tensor([P, T_], DT.uint8))
        nsm_sb = e(nc.sbuf_tensor([P, T_], DT.uint8))
        m_sb = e(nc.sbuf_tensor([P, T_], DT.uint8))
        xt_a = e(nc.sbuf_tensor([P, CE], DT.float32))
        xt_b = e(nc.sbuf_tensor([P, CE], DT.float32))
        aown = e(nc.sbuf_tensor([P, H], DT.float32))
        afull = e(nc.sbuf_tensor([P, L], DT.float32))
        sc = e(nc.sbuf_tensor([P, H * L], DT.float32))
        R = e(nc.sbuf_tensor([P, H * L], DT.float32))
        ltt = e(nc.sbuf_tensor([P, H * L], DT.float16))
        bp_own = e(nc.sbuf_tensor([P, H], DT.uint8))
        cand = e(nc.sbuf_tensor([P, H], DT.float32))
        ring = e(nc.sbuf_tensor([P, T_ * L], DT.uint8))
        paths = e(nc.sbuf_tensor([P, T_], DT.float32))
        outi = e(nc.sbuf_tensor([P, T_], DT.int32))
        lt32 = e(nc.sbuf_tensor([P, L], DT.float32))
        junk = e(nc.sbuf_tensor([P, L], DT.float32))
        tbl_sem = e(nc.semaphore("tbl_sem"))
        xa_sem = e(nc.semaphore("xa_sem"))
        xb_sem = e(nc.semaphore("xb_sem"))
        out_sem = e(nc.semaphore("out_sem"))
        dve_sem = e(nc.semaphore("dve_sem"))

        xt_ab = [xt_a, xt_b]
        trep3 = trep_sb[:].rearrange("p (j i) -> p j i", i=L)
        sc3 = sc[:].rearrange("p (j i) -> p j i", i=L)
        R3 = R[:].rearrange("p (j i) -> p j i", i=L)
        lt3 = ltt[:].rearrange("p (j i) -> p j i", i=L)
        Mv = R3[:, :, L - 1 : L]
        afull_b = afull[:].unsqueeze(1).broadcast_to([P, H, L])
        ring3 = ring[:].rearrange("p (t l) -> p t l", l=L)
        HH = H // 2  # half of the own pages, for lt/reduce splitting

        with nc.Block() as block:
            marks = {}
            total = [0]

            @block.vector
            def _(v):
                n = [0]

                def S(inst):
                    inst.then_inc(dve_sem, 1)
                    n[0] += 1
                    return n[0]

                def W(k):
                    if k is not None:
                        v.wait_ge(dve_sem, k)

                v.wait_ge(tbl_sem, 16 * 8)
                v.wait_ge(xa_sem, 16)  # chunk 0
                i_aown = S(v.tensor_copy(aown[:], xt_a[:, 0:H]))
                W(i_aown)
                S(v.stream_shuffle(afull[:, 0:H], aown[:], EVEN))
                i_shufO = S(v.stream_shuffle(afull[:, H:L], aown[:], ODD))

                # software-pipelined forward loop.
                # iteration t emits: alpha-chain(t), lt halves + reduce halves
                # of (t-1), ring shuffles of (t-2). Producer indices tracked
                # for exact wait targets.
                idx_pred = {}
                idx_shufO = {1: i_shufO}
                idx_scan = {}
                idx_ltb = {}
                idx_redb = {}

                for t in range(1, T_ + 2):
                    cur = t <= T_ - 1
                    c = t // CHUNK if cur else 0
                    u = t % CHUNK
                    xt = xt_ab[c % 2]
                    if cur and u == 0:
                        # first step of chunk c: ensure its DMA landed
                        # (issued ~CHUNK steps ago -> free wait)
                        if c % 2 == 0:
                            v.wait_ge(xa_sem, 16 * (c // 2 + 1))
                        else:
                            v.wait_ge(xb_sem, 16 * ((c - 1) // 2 + 1))

                    # --- alpha chain of t (+ interleaved fillers) ---
                    if cur and t > 1:
                        W(idx_pred[t - 1])
                        S(v.stream_shuffle(afull[:, 0:H], aown[:], EVEN))
                        idx_shufO[t] = S(v.stream_shuffle(afull[:, H:L], aown[:], ODD))
                    # filler: lt first half of t-1
                    if 1 <= t - 1 <= T_ - 1:
                        W(idx_scan[t - 1])
                        idx_lta = S(v.tensor_tensor(
                            out=lt3[:, 0:HH, :], in0=R3[:, 0:HH, :],
                            in1=Mv[:, 0:HH, :].broadcast_to([P, HH, L]), op=A.is_lt))
                    if cur:
                        W(idx_shufO[t])
                        i_add = S(v.tensor_tensor(out=sc3, in0=afull_b, in1=trep3, op=A.add))
                    # filler: lt second half of t-1
                    if 1 <= t - 1 <= T_ - 1:
                        idx_ltb[t - 1] = S(v.tensor_tensor(
                            out=lt3[:, HH:H, :], in0=R3[:, HH:H, :],
                            in1=Mv[:, HH:H, :].broadcast_to([P, HH, L]), op=A.is_lt))
                    if cur:
                        W(i_add)
                        idx_scan[t] = S(v.tensor_tensor_scan(
                            out=R[:], data0=rstv_sb[:], data1=sc[:],
                            initial=0.0, op0=A.add, op1=A.max))
                    # filler: ring shuffles of t-2
                    if 1 <= t - 2 <= T_ - 1:
                        W(idx_redb[t - 2])
                        S(v.stream_shuffle(ring3[:, t - 2, 0:H], bp_own[:], EVEN))
                        S(v.stream_shuffle(ring3[:, t - 2, H:L], bp_own[:], ODD))
                    if cur:
                        W(idx_scan[t])
                        i_cand = S(v.tensor_tensor(
                            out=cand[:].unsqueeze(2), in0=Mv,
                            in1=xt[:, u * H : (u + 1) * H].unsqueeze(2), op=A.add))
                    # filler: reduce first half of t-1
                    if 1 <= t - 1 <= T_ - 1:
                        W(idx_ltb[t - 1])
                        with nc.allow_low_precision(reason="bp count <= 32, exact in u8"):
                            S(v.tensor_reduce(
                                out=bp_own[:, 0:HH], in_=lt3[:, 0:HH, :], axis=AX.X, op=A.add))
                    if cur:
                        W(i_cand)
                        inst = v.copy_predicated(
                            out=aown[:],
                            mask=sm_sb[:, t : t + 1].broadcast_to([P, H]),
                            data=cand[:])
                        idx_pred[t] = S(inst)
                        if u == CHUNK - 1:
                            # chunk c fully consumed by DVE at this point
                            marks[c] = n[0]
                    # tail filler: reduce second half of t-1
                    if 1 <= t - 1 <= T_ - 1:
                        with nc.allow_low_precision(reason="bp count <= 32, exact in u8"):
                            idx_redb[t - 1] = S(v.tensor_reduce(
                                out=bp_own[:, HH:H], in_=lt3[:, HH:H, :], axis=AX.X, op=A.add))

                # --- identity backpointers on masked steps ---
                W(n[0])
                for l_ in range(L):
                    S(v.copy_predicated(
                        out=ring3[:, 1:, l_],
                        mask=nsm_sb[:, 1:],
                        data=iotu_sb[:, l_ : l_ + 1].broadcast_to([P, T_ - 1])))

                # --- final argmax: paths[:, T-1] (first argmax of afull) ---
                # rebuild afull: the loop's last shuffles ran before pred(T-1),
                # so len==T sequences have a stale second... both halves.
                W(idx_pred[T_ - 1])
                S(v.stream_shuffle(afull[:, 0:H], aown[:], EVEN))
                S(v.stream_shuffle(afull[:, H:L], aown[:], ODD))
                W(n[0])
                S(v.tensor_tensor_scan(
                    out=lt32[:], data0=rstv_sb[:, 0:L], data1=afull[:],
                    initial=0.0, op0=A.add, op1=A.max))
                W(n[0])
                S(v.tensor_tensor(
                    out=junk[:], in0=lt32[:],
                    in1=lt32[:, L - 1 : L].broadcast_to([P, L]), op=A.is_lt))
                W(n[0])
                S(v.tensor_reduce(
                    out=paths[:, T_ - 1 : T_], in_=junk[:], axis=AX.X, op=A.add))

                # --- backtrace ---
                for t in range(T_ - 2, -1, -1):
                    W(n[0])
                    S(v.scalar_tensor_tensor(
                        out=junk[:],
                        in0=iotf_sb[:],
                        scalar=paths[:, t + 1 : t + 2],
                        in1=ring3[:, t + 1, :],
                        op0=A.is_equal,
                        op1=A.mult,
                        accum_out=paths[:, t : t + 1]))

                # --- mask (MASK_ID = 0), cast int32, signal output DMA ---
                W(n[0])
                S(v.tensor_tensor(out=paths[:], in0=paths[:], in1=m_sb[:], op=A.mult))
                W(n[0])
                S(v.tensor_copy(outi[:], paths[:]))
                total[0] = n[0]
                v.wait_ge(out_sem, 16)

            @block.gpsimd
            def _(g):
                g.dma_start(trep_sb[:], trep[:]).then_inc(tbl_sem, 16)
                g.dma_start(rstv_sb[:], rstv[:]).then_inc(tbl_sem, 16)
                g.dma_start(iotf_sb[:], iotf[:]).then_inc(tbl_sem, 16)
                g.dma_start(iotu_sb[:], iotu[:]).then_inc(tbl_sem, 16)
                g.dma_start(sm_sb[:], smt[:]).then_inc(tbl_sem, 16)
                g.dma_start(nsm_sb[:], nsmt[:]).then_inc(tbl_sem, 16)
                g.dma_start(m_sb[:], mt[:]).then_inc(tbl_sem, 16)
                g.dma_start(iotu_sb[:], iotu[:]).then_inc(tbl_sem, 16)
                for c in range(nch):
                    if c >= 2:
                        g.wait_ge(dve_sem, marks[c - 2])
                    g.dma_start(
                        xt_ab[c % 2][:], x[:, c * CE : (c + 1) * CE]
                    ).then_inc(xa_sem if c % 2 == 0 else xb_sem, 16)
                g.wait_ge(dve_sem, total[0])
                g.dma_start(pout[:], outi[:]).then_inc(out_sem, 16)

    return nc


def make_tables(trans_params, T_=T):
    tT = np.ascontiguousarray(np.asarray(trans_params, np.float32).T)  # [j, i]
    trep = np.zeros((P, H, L), np.float32)
    trep[0::2] = tT[None, 0:H, :]
    trep[1::2] = tT[None, H:L, :]
    rstv = np.zeros((P, H, L), np.float32)
    rstv[:, :, 0] = NEG
    iotf = np.tile(np.arange(L, dtype=np.float32), (P, 1))
    iotu = np.tile(np.arange(L, dtype=np.uint8), (P, 1))
    return trep.reshape(P, H * L), rstv.reshape(P, H * L), iotf, iotu


def prepare_in_maps(np_inputs, T_=T):
    inputs = np.asarray(np_inputs["inputs"], dtype=np.float32)
    seq_lengths = np.asarray(np_inputs["seq_lengths"], dtype=np.int32)
    trans_params = np.asarray(np_inputs["trans_params"], dtype=np.float32)

    trep, rstv, iotf, iotu = make_tables(trans_params, T_=T_)

    t_idx = np.arange(T_, dtype=np.int64)
    in_maps = []
    for k in range(NCORES):
        xs = inputs[k * SEQ : (k + 1) * SEQ]          # [SEQ, T, L]
        ls = seq_lengths[k * SEQ : (k + 1) * SEQ]     # [SEQ]
        # pair-split emissions: partition 2s = j 0..15, 2s+1 = j 16..31
        xo = np.empty((P, T_, H), np.float32)
        xo[0::2] = xs[:, :, 0:H]
        xo[1::2] = xs[:, :, H:L]
        m = (t_idx[None, :] < ls[:, None])            # [SEQ, T]
        sm = m & (t_idx[None, :] >= 1)
        mm = np.repeat(m, 2, axis=0).astype(np.uint8)
        smm = np.repeat(sm, 2, axis=0).astype(np.uint8)
        nsmm = (1 - smm).astype(np.uint8)
        in_maps.append({
            "x": xo.reshape(P, T_ * H),
            "trep": trep,
            "rstv": rstv,
            "iotf": iotf,
            "iotu": iotu,
            "smt": smm,
            "nsmt": nsmm,
            "mt": mm,
        })
    return in_maps, None


def assemble_output(results):
    paths = np.stack(
        [results[k]["paths"][0::2, :] for k in range(NCORES)], axis=0
    )
    return paths.reshape(B, T).astype(np.int32)


def kernel(inputs, seq_lengths, trans_params):
    nc = build_program()
    in_maps, _ = prepare_in_maps(
        {
            "inputs": inputs,
            "seq_lengths": seq_lengths,
            "trans_params": trans_params,
        }
    )
    res = run_bass_kernel_spmd(nc, in_maps, list(range(NCORES)))
    return assemble_output(res.results)
